# revision 6
# baseline (speedup 1.0000x reference)
"""Distributed Trainium2 kernel for GNN message passing (COO SpMM + dense head).

out = relu((A @ x) @ W[:128] + x @ W[128:])   with A given as COO (rows, cols, vals)

Strategy (8 NeuronCores, SPMD single graph):
  - Rows (destinations) sharded across cores: core c owns rows [c*12500, (c+1)*12500).
  - x replicated to every core's DRAM (bf16) via its input map; no collectives.
  - SpMM = hardware gather + SEGMENT-SUM VIA TENSOR-ENGINE MATMULS (no SWDGE
    scatter at all -- scatter-add descriptor generation was the baseline's
    dominant Pool-engine cost at ~6ns/descriptor):
      * edges sorted by (col-chunk k, row-group g = r//128, row); per (k,g)
        cell the edge count is padded to a shared 128-aligned capacity across
        cores (SPMD: one program). Pad slots hold gather idx -1 (skipped when
        trailing in a call) or 0 (mid-call; S column is zero either way).
      * gather calls are 1024-slot windows of each chunk's stream -- per-call
        fixed overhead on the Q7 descgen cores is ~600ns, so few big calls
        (98) beat per-cell calls (392).
      * per 128-edge subtile: one bf16 matmul  psum_g += msgs^T @ S_sub where
        S_sub[i, j] = val_i * onehot(r_i - 128g == j) is HOST-precomputed bf16
        (values folded in -> no vector work in the inner loop). psum_g
        accumulates the whole group's 4 chunk-cells (cells located inside the
        big gather windows by static offset arithmetic), then one scalar
        activation copies it (cast bf16) into an SBUF-resident hT arena
        [128 feat x 12800 rows]. h never touches DRAM.
  - Dense head overlapped with SpMM: every 4 groups, outT = relu(W1^T @ hT +
    W2^T @ xT) with N=512 matmuls (W stationary), relu on ScalarE, contiguous
    store of outT [128 x 12800]; host transposes at the end.
"""

import sys

if "/opt/trn_rl_repo" not in sys.path:
    sys.path.insert(0, "/opt/trn_rl_repo")

import numpy as np
import ml_dtypes

BF16 = ml_dtypes.bfloat16
FP8 = ml_dtypes.float8_e4m3

N_NODES = 100000
N_EDGES = 600000
D = 128
OUT = 128
P = 128
NCORES = 8
RPC = N_NODES // NCORES          # 12500 rows per core
NCHUNK = 4
CHUNK = N_NODES // NCHUNK        # 25000 (< 32768 so int16 gather idx works)
NG = (RPC + P - 1) // P          # 98 row-groups of 128 rows
RPAD = 12800                     # 25 head batches x 512 rows
CALL = 1024                      # gather slots per SWDGE call (ring limit)

_compiled = {}


def _prep(adj_rows, adj_cols, adj_vals):
    """Per-core uniform-shape gather idx + segment-matrix streams.

    Edges of core c sorted by (chunk=col//25000, r). Cell (k,g) capacity =
    max over cores, rounded up to 128 (subtile size). Four chunk-major
    streams concatenated; stream k starts at GO[k] (1024-aligned). Streams:
      gi : int16 gather indices (col % 25000); pads are 0 (mid-call) or -1
           (trailing in the last call of a stream)
      S  : bf16 [128, T]; edge at global slot i -> S[i%128,
           (i//128)*128 + (r - 128g)] = val. Pad slots: zero columns.
    """
    rows = np.asarray(adj_rows).astype(np.int64)
    cols = np.asarray(adj_cols).astype(np.int64)
    vals = np.asarray(adj_vals).astype(np.float32)

    per_core = []
    counts = np.zeros((NCORES, NCHUNK * NG), np.int64)
    for c in range(NCORES):
        m = (rows >= c * RPC) & (rows < (c + 1) * RPC)
        r = rows[m] - c * RPC
        co = cols[m]
        v = vals[m]
        ch = co // CHUNK
        o = np.lexsort((r, ch))
        r, co, v, ch = r[o], co[o], v[o], ch[o]
        cell = ch * NG + (r >> 7)
        counts[c] = np.bincount(cell, minlength=NCHUNK * NG)
        per_core.append((r, co, v, cell))

    caps = ((counts.max(axis=0) + 127) // 128) * 128
    caps = np.maximum(caps, 128).reshape(NCHUNK, NG)    # [k, g]
    sk = caps.sum(axis=1)                               # stream k size
    go = np.concatenate([[0], np.cumsum(((sk + CALL - 1) // CALL) * CALL)])
    T = int(go[-1])
    # stream-local cell offsets
    cell_off = np.zeros((NCHUNK, NG), np.int64)
    cell_off[:, 1:] = np.cumsum(caps, axis=1)[:, :-1]
    # global slot offset per cell, flattened in (k, g) order
    gcell_off = (cell_off + go[:-1, None]).reshape(-1)

    gi_w = np.zeros((NCORES, P, T // 16), np.int16)
    s_w = np.zeros((NCORES, P, T), FP8)
    v_w = np.zeros((NCORES, P, T // P), BF16)
    for c in range(NCORES):
        r, co, v, cell = per_core[c]
        n = len(r)
        starts = np.concatenate([[0], np.cumsum(counts[c])])
        pos = gcell_off[cell] + (np.arange(n) - starts[cell])
        gi = np.zeros(T, np.int16)
        for k in range(NCHUNK):                 # trailing pads of each stream
            gi[int(go[k]) + int(sk[k]):int(go[k + 1])] = -1
        gi[pos] = (co % CHUNK).astype(np.int16)
        gi_w[c] = np.tile(gi.reshape(-1, 16).T, (8, 1))
        S = np.zeros((P, T), np.float32)
        S[pos % P, (pos // P) * P + (r - ((r >> 7) << 7))] = 1.0
        s_w[c] = S.astype(FP8)
        V = np.zeros((P, T // P), np.float32)
        V[pos % P, pos // P] = v
        v_w[c] = V.astype(BF16)

    key = tuple(int(x) for x in caps.reshape(-1))
    return key, gi_w, s_w, v_w


def _build(key):
    from concourse import bass, mybir, tile, bacc

    f32 = mybir.dt.float32
    bf16 = mybir.dt.bfloat16
    i16 = mybir.dt.int16
    fp8 = mybir.dt.float8e4
    relu = mybir.ActivationFunctionType.Relu
    copyf = mybir.ActivationFunctionType.Copy

    caps = np.asarray(key, np.int64).reshape(NCHUNK, NG)
    sk = caps.sum(axis=1)
    go = np.concatenate([[0], np.cumsum(((sk + CALL - 1) // CALL) * CALL)])
    T = int(go[-1])
    cell_off = np.zeros((NCHUNK, NG), np.int64)
    cell_off[:, 1:] = np.cumsum(caps, axis=1)[:, :-1]

    # S mega-tile windows: per (4-group block m, stream k)
    MBLK = 4
    nmega = (NG + MBLK - 1) // MBLK
    def mwin(m, k):
        ge = min((m + 1) * MBLK, NG) - 1
        lo = int(cell_off[k, m * MBLK])
        hi = int(cell_off[k, ge] + caps[k, ge])
        return lo, hi
    mega_max = max(mwin(m, k)[1] - mwin(m, k)[0]
                   for m in range(nmega) for k in range(NCHUNK))

    nc = bacc.Bacc("TRN2", target_bir_lowering=False, debug=False,
                   num_swdge_queues=4)

    x_d = nc.dram_tensor("x", [N_NODES, D], bf16, kind="ExternalInput")
    xT_d = nc.dram_tensor("xlocT", [D, RPAD], bf16, kind="ExternalInput")
    w_d = nc.dram_tensor("W", [2 * D, OUT], bf16, kind="ExternalInput")
    gi_d = nc.dram_tensor("gidx", [P, T // 16], i16, kind="ExternalInput")
    s_d = nc.dram_tensor("smat", [P, T], fp8, kind="ExternalInput")
    v_d = nc.dram_tensor("vals", [P, T // P], bf16, kind="ExternalInput")
    outT_d = nc.dram_tensor("outT", [OUT, RPAD], bf16, kind="ExternalOutput")

    with tile.TileContext(nc) as tc:
        with tc.tile_pool(name="const", bufs=1) as constp, \
             tc.tile_pool(name="smega", bufs=8) as smp, \
             tc.tile_pool(name="mess", bufs=12) as mvp, \
             tc.tile_pool(name="outb", bufs=2) as outp, \
             tc.tile_pool(name="psseg", bufs=3, space="PSUM") as segp, \
             tc.tile_pool(name="pshead", bufs=2, space="PSUM") as headp:

            # gather idx arena (whole stream resident)
            gi_t = constp.tile([P, T // 16], i16)
            qn = T // 16 // 4
            for q in range(4):
                q0 = q * qn
                q1 = (q + 1) * qn if q < 3 else T // 16
                nc.sync.dma_start(out=gi_t[:, q0:q1], in_=gi_d[:, q0:q1])

            val_t = constp.tile([P, T // P], bf16)
            nc.sync.dma_start(out=val_t[:], in_=v_d[:])
            w1 = constp.tile([D, OUT], bf16)
            nc.scalar.dma_start(out=w1[:], in_=w_d[:D, :])
            w2 = constp.tile([D, OUT], bf16)
            nc.scalar.dma_start(out=w2[:], in_=w_d[D:, :])
            xta = constp.tile([P, RPAD], bf16)
            for q in range(4):
                nc.scalar.dma_start(out=xta[:, q * 3200:(q + 1) * 3200],
                                    in_=xT_d[:, q * 3200:(q + 1) * 3200])

            arena = constp.tile([P, RPAD], bf16)
            # groups only cover [0, NG*128); memset the tail once
            nc.vector.memset(arena[:, NG * P:], 0.0)

            # pre-zero the gather buffers: trailing-negative pad slots are
            # skipped by descgen and would otherwise read uninitialized SBUF
            # (NaN * 0 = NaN in the matmul)
            for b in range(12):
                mv = mvp.tile([P, CALL // P, D], bf16, tag="mv")
                nc.vector.memset(mv[:], 0.0)

            ncalls = [(int(sk[k]) + CALL - 1) // CALL for k in range(NCHUNK)]
            callidx = [0, 0, 0, 0]
            mvtiles = [dict() for _ in range(NCHUNK)]
            smtiles = [None] * NCHUNK
            smlo = [0] * NCHUNK
            qrr = 0

            for g in range(NG):
                m = g // MBLK
                if g % MBLK == 0:
                    for k in range(NCHUNK):
                        lo, hi = mwin(m, k)
                        sm = smp.tile([P, mega_max], fp8, tag=f"sm{k}")
                        nc.sync.dma_start(
                            out=sm[:, :hi - lo],
                            in_=s_d[:, int(go[k]) + lo:int(go[k]) + hi])
                        smtiles[k], smlo[k] = sm, lo

                # issue gather calls covering this group's cells
                for k in range(NCHUNK):
                    need = int(cell_off[k, g] + caps[k, g])
                    while callidx[k] * CALL < need:
                        w = callidx[k]
                        nn = min(CALL, int(sk[k]) - w * CALL)
                        nn = ((nn + 127) // 128) * 128
                        mv = mvp.tile([P, CALL // P, D], bf16, tag="mv")
                        b0 = int(go[k]) + w * CALL
                        nc.gpsimd.dma_gather(
                            mv[:, :nn // P, :],
                            x_d[k * CHUNK:(k + 1) * CHUNK, :],
                            gi_t[:, b0 // 16:(b0 + nn) // 16], nn, nn, D,
                            queue_num=1 + qrr % 3)
                        qrr += 1
                        nc.vector.tensor_tensor(
                            out=mv[:, :nn // P, :], in0=mv[:, :nn // P, :],
                            in1=val_t[:, b0 // P:(b0 + nn) // P, None
                                      ].to_broadcast([P, nn // P, D]),
                            op=mybir.AluOpType.mult)
                        mvtiles[k][w] = mv
                        if w >= 3:
                            mvtiles[k].pop(w - 3, None)
                        callidx[k] += 1

                psg = segp.tile([P, P], f32, tag="ps")
                nsub_tot = int(caps[:, g].sum()) // P
                sdone = 0
                for k in range(NCHUNK):
                    for s in range(int(caps[k, g]) // P):
                        q = int(cell_off[k, g]) + s * P
                        mv = mvtiles[k][q // CALL]
                        nc.tensor.matmul(
                            psg[:], mv[:, (q % CALL) // P, :],
                            smtiles[k][:, q - smlo[k]:q - smlo[k] + P],
                            start=(sdone == 0), stop=(sdone == nsub_tot - 1))
                        sdone += 1
                nc.scalar.activation(arena[:, g * P:(g + 1) * P], psg[:], copyf)

                if g % 4 == 3:
                    c0 = (g // 4) * 512
                    ph = headp.tile([P, 512], f32, tag="ph")
                    nc.tensor.matmul(ph[:], w1[:], arena[:, c0:c0 + 512],
                                     start=True, stop=False)
                    nc.tensor.matmul(ph[:], w2[:], xta[:, c0:c0 + 512],
                                     start=False, stop=True)
                    ob = outp.tile([P, 512], bf16, tag="ob")
                    nc.scalar.activation(ob[:], ph[:], relu)
                    nc.scalar.dma_start(out=outT_d[:, c0:c0 + 512], in_=ob[:])

            # final head batch: groups 96..97 plus zero tail (12288..12800)
            c0 = (NG // 4) * 512
            assert c0 == 12288
            ph = headp.tile([P, 512], f32, tag="ph")
            nc.tensor.matmul(ph[:], w1[:], arena[:, c0:c0 + 512],
                             start=True, stop=False)
            nc.tensor.matmul(ph[:], w2[:], xta[:, c0:c0 + 512],
                             start=False, stop=True)
            ob = outp.tile([P, 512], bf16, tag="ob")
            nc.scalar.activation(ob[:], ph[:], relu)
            nc.scalar.dma_start(out=outT_d[:, c0:c0 + 512], in_=ob[:])

    nc.compile()
    return nc


def _get_nc(key):
    nc = _compiled.get(key)
    if nc is None:
        nc = _build(key)
        _compiled[key] = nc
    return nc


def _make_in_maps(x, W, gi_w, s_w, v_w):
    x = np.asarray(x, np.float32)
    xb = x.astype(BF16)
    Wb = np.ascontiguousarray(np.asarray(W, np.float32).astype(BF16))
    in_maps = []
    for c in range(NCORES):
        xloc = np.zeros((D, RPAD), BF16)
        xloc[:, :RPC] = xb[c * RPC:(c + 1) * RPC].T
        in_maps.append({
            "x": xb,
            "xlocT": np.ascontiguousarray(xloc),
            "W": Wb,
            "gidx": gi_w[c],
            "smat": s_w[c],
            "vals": v_w[c],
        })
    return in_maps


def _install_trace_shims():
    """Make trace=True work in this container: provide antenv.axon_hooks
    (ctypes NTFF profiling via the axon PJRT .so) and stub the artifact
    upload (no bucket access here)."""
    import contextlib
    import ctypes
    import types

    try:
        import antenv.axon_hooks  # noqa: F401
        has_hooks = True
    except ImportError:
        has_hooks = False
    if not has_hooks:
        so_path = "/opt/axon/libaxon_pjrt.so"
        lib = ctypes.CDLL(so_path)
        if hasattr(lib, "axon_start_nrt_profile"):
            lib.axon_start_nrt_profile.argtypes = [
                ctypes.POINTER(ctypes.c_int64), ctypes.c_size_t]
            lib.axon_start_nrt_profile.restype = ctypes.c_int64
            lib.axon_stop_nrt_profile.argtypes = [ctypes.c_char_p]
            lib.axon_stop_nrt_profile.restype = ctypes.c_int64

            @contextlib.contextmanager
            def _hook(output_dir, device_ids):
                import jax
                jax.devices()
                if device_ids:
                    ids = (ctypes.c_int64 * len(device_ids))(*device_ids)
                    rc = lib.axon_start_nrt_profile(ids, len(device_ids))
                else:
                    rc = lib.axon_start_nrt_profile(None, 0)
                if rc != 0:
                    raise RuntimeError(f"axon_start_nrt_profile rc={rc}")
                try:
                    yield
                finally:
                    n = lib.axon_stop_nrt_profile(str(output_dir).encode())
                    if n <= 0:
                        print(f"ntff profile: rc={n} (no files?) at {output_dir}")

            mod = types.ModuleType("antenv.axon_hooks")
            mod.get_axon_ntff_profile_hook = lambda: _hook
            mod.set_axon_ntff_profile_hook = lambda h: None
            sys.modules["antenv.axon_hooks"] = mod

    import concourse.bass_utils as bu
    bu.upload_artifacts = lambda tmpdir: f"local:{tmpdir}"


def _run(x, adj_rows, adj_cols, adj_vals, W, trace=False):
    from concourse.bass_utils import run_bass_kernel_spmd
    if trace:
        try:
            _install_trace_shims()
        except Exception as e:  # tracing is best-effort
            print("trace shim install failed:", e)
    key, gi_w, s_w, v_w = _prep(adj_rows, adj_cols, adj_vals)
    nc = _get_nc(key)
    in_maps = _make_in_maps(x, W, gi_w, s_w, v_w)
    res = run_bass_kernel_spmd(nc, in_maps, list(range(NCORES)), trace=trace)
    out = np.concatenate(
        [np.asarray(res.results[c]["outT"])[:, :RPC].T.astype(np.float32)
         for c in range(NCORES)],
        axis=0)
    return np.ascontiguousarray(out, dtype=np.float32), res


def kernel(x, adj_rows, adj_cols, adj_vals, W):
    out, _ = _run(x, adj_rows, adj_cols, adj_vals, W, trace=False)
    return out


# revision 7
# speedup vs baseline: 1.2215x; 1.2215x over previous
"""Distributed Trainium2 kernel for GNN message passing (COO SpMM + dense head).

out = relu((A @ x) @ W[:128] + x @ W[128:])   with A given as COO (rows, cols, vals)

Strategy (8 NeuronCores, SPMD single graph):
  - Rows (destinations) sharded across cores: core c owns rows [c*12500, (c+1)*12500).
  - x replicated to every core's DRAM (bf16) via its input map; no collectives.
  - SpMM = hardware gather + SEGMENT-SUM VIA TENSOR-ENGINE MATMULS (no SWDGE
    scatter at all -- scatter-add descriptor generation was the baseline's
    dominant Pool-engine cost at ~6ns/descriptor):
      * edges sorted by (col-chunk k, row-group g = r//128, row); per (k,g)
        cell the edge count is padded to a shared 128-aligned capacity across
        cores (SPMD: one program). Pad slots hold gather idx -1 (skipped when
        trailing in a call) or 0 (mid-call; S column is zero either way).
      * gather calls are 1024-slot windows of each chunk's stream -- per-call
        fixed overhead on the Q7 descgen cores is ~600ns, so few big calls
        (98) beat per-cell calls (392).
      * per 128-edge subtile: one bf16 matmul  psum_g += msgs^T @ S_sub where
        S_sub[i, j] = val_i * onehot(r_i - 128g == j) is HOST-precomputed bf16
        (values folded in -> no vector work in the inner loop). psum_g
        accumulates the whole group's 4 chunk-cells (cells located inside the
        big gather windows by static offset arithmetic), then one scalar
        activation copies it (cast bf16) into an SBUF-resident hT arena
        [128 feat x 12800 rows]. h never touches DRAM.
  - Dense head overlapped with SpMM: every 4 groups, outT = relu(W1^T @ hT +
    W2^T @ xT) with N=512 matmuls (W stationary), relu on ScalarE, contiguous
    store of outT [128 x 12800]; host transposes at the end.
"""

import sys

if "/opt/trn_rl_repo" not in sys.path:
    sys.path.insert(0, "/opt/trn_rl_repo")

import numpy as np
import ml_dtypes

BF16 = ml_dtypes.bfloat16
FP8 = ml_dtypes.float8_e4m3

N_NODES = 100000
N_EDGES = 600000
D = 128
OUT = 128
P = 128
NCORES = 8
RPC = N_NODES // NCORES          # 12500 rows per core
NCHUNK = 4
CHUNK = N_NODES // NCHUNK        # 25000 (< 32768 so int16 gather idx works)
NG = (RPC + P - 1) // P          # 98 row-groups of 128 rows
RPAD = 12800                     # 25 head batches x 512 rows
CALL = 1024                      # gather slots per SWDGE call (ring limit)

_compiled = {}


def _prep(adj_rows, adj_cols, adj_vals):
    """Per-core uniform-shape gather idx + segment-matrix streams.

    Edges of core c sorted by (chunk=col//25000, r). Cell (k,g) capacity =
    max over cores, rounded up to 128 (subtile size). Four chunk-major
    streams concatenated; stream k starts at GO[k] (1024-aligned). Streams:
      gi : int16 gather indices (col % 25000); pads are 0 (mid-call) or -1
           (trailing in the last call of a stream)
      S  : bf16 [128, T]; edge at global slot i -> S[i%128,
           (i//128)*128 + (r - 128g)] = val. Pad slots: zero columns.
    """
    rows = np.asarray(adj_rows).astype(np.int64)
    cols = np.asarray(adj_cols).astype(np.int64)
    vals = np.asarray(adj_vals).astype(np.float32)

    per_core = []
    counts = np.zeros((NCORES, NCHUNK * NG), np.int64)
    for c in range(NCORES):
        m = (rows >= c * RPC) & (rows < (c + 1) * RPC)
        r = rows[m] - c * RPC
        co = cols[m]
        v = vals[m]
        ch = co // CHUNK
        o = np.lexsort((r, ch))
        r, co, v, ch = r[o], co[o], v[o], ch[o]
        cell = ch * NG + (r >> 7)
        counts[c] = np.bincount(cell, minlength=NCHUNK * NG)
        per_core.append((r, co, v, cell))

    caps = ((counts.max(axis=0) + 127) // 128) * 128
    caps = np.maximum(caps, 128).reshape(NCHUNK, NG)    # [k, g]
    sk = caps.sum(axis=1)                               # stream k size
    go = np.concatenate([[0], np.cumsum(((sk + CALL - 1) // CALL) * CALL)])
    T = int(go[-1])
    # stream-local cell offsets
    cell_off = np.zeros((NCHUNK, NG), np.int64)
    cell_off[:, 1:] = np.cumsum(caps, axis=1)[:, :-1]
    # global slot offset per cell, flattened in (k, g) order
    gcell_off = (cell_off + go[:-1, None]).reshape(-1)

    gi_w = np.zeros((NCORES, P, T // 16), np.int16)
    s_w = np.zeros((NCORES, P, T), FP8)
    v_w = np.zeros((NCORES, P, T // P), BF16)
    for c in range(NCORES):
        r, co, v, cell = per_core[c]
        n = len(r)
        starts = np.concatenate([[0], np.cumsum(counts[c])])
        pos = gcell_off[cell] + (np.arange(n) - starts[cell])
        gi = np.zeros(T, np.int16)
        for k in range(NCHUNK):                 # trailing pads of each stream
            gi[int(go[k]) + int(sk[k]):int(go[k + 1])] = -1
        gi[pos] = (co % CHUNK).astype(np.int16)
        gi_w[c] = np.tile(gi.reshape(-1, 16).T, (8, 1))
        S = np.zeros((P, T), np.float32)
        S[pos % P, (pos // P) * P + (r - ((r >> 7) << 7))] = 1.0
        s_w[c] = S.astype(FP8)
        V = np.zeros((P, T // P), np.float32)
        V[pos % P, pos // P] = v
        v_w[c] = V.astype(BF16)

    key = tuple(int(x) for x in caps.reshape(-1))
    return key, gi_w, s_w, v_w


def _build(key):
    from concourse import bass, mybir, tile, bacc

    f32 = mybir.dt.float32
    bf16 = mybir.dt.bfloat16
    i16 = mybir.dt.int16
    fp8 = mybir.dt.float8e4
    relu = mybir.ActivationFunctionType.Relu
    copyf = mybir.ActivationFunctionType.Copy

    caps = np.asarray(key, np.int64).reshape(NCHUNK, NG)
    sk = caps.sum(axis=1)
    go = np.concatenate([[0], np.cumsum(((sk + CALL - 1) // CALL) * CALL)])
    T = int(go[-1])
    cell_off = np.zeros((NCHUNK, NG), np.int64)
    cell_off[:, 1:] = np.cumsum(caps, axis=1)[:, :-1]

    # S mega-tile windows: per (4-group block m, stream k)
    MBLK = 4
    nmega = (NG + MBLK - 1) // MBLK
    def mwin(m, k):
        ge = min((m + 1) * MBLK, NG) - 1
        lo = int(cell_off[k, m * MBLK])
        hi = int(cell_off[k, ge] + caps[k, ge])
        return lo, hi
    mega_max = max(mwin(m, k)[1] - mwin(m, k)[0]
                   for m in range(nmega) for k in range(NCHUNK))

    nc = bacc.Bacc("TRN2", target_bir_lowering=False, debug=False,
                   num_swdge_queues=4)

    x_d = nc.dram_tensor("x", [N_NODES, D], bf16, kind="ExternalInput")
    xT_d = nc.dram_tensor("xlocT", [D, RPAD], bf16, kind="ExternalInput")
    w_d = nc.dram_tensor("W", [2 * D, OUT], bf16, kind="ExternalInput")
    gi_d = nc.dram_tensor("gidx", [P, T // 16], i16, kind="ExternalInput")
    s_d = nc.dram_tensor("smat", [P, T], fp8, kind="ExternalInput")
    v_d = nc.dram_tensor("vals", [P, T // P], bf16, kind="ExternalInput")
    outT_d = nc.dram_tensor("outT", [OUT, RPAD], bf16, kind="ExternalOutput")

    with tile.TileContext(nc) as tc:
        with tc.tile_pool(name="const", bufs=1) as constp, \
             tc.tile_pool(name="smega", bufs=8) as smp, \
             tc.tile_pool(name="mess", bufs=12) as mvp, \
             tc.tile_pool(name="outb", bufs=2) as outp, \
             tc.tile_pool(name="psseg", bufs=3, space="PSUM") as segp, \
             tc.tile_pool(name="pshead", bufs=2, space="PSUM") as headp:

            # gather idx arena (whole stream resident)
            gi_t = constp.tile([P, T // 16], i16)
            qn = T // 16 // 4
            for q in range(4):
                q0 = q * qn
                q1 = (q + 1) * qn if q < 3 else T // 16
                nc.sync.dma_start(out=gi_t[:, q0:q1], in_=gi_d[:, q0:q1])

            val_t = constp.tile([P, T // P], bf16)
            nc.sync.dma_start(out=val_t[:], in_=v_d[:])
            w1 = constp.tile([D, OUT], bf16)
            nc.scalar.dma_start(out=w1[:], in_=w_d[:D, :])
            w2 = constp.tile([D, OUT], bf16)
            nc.scalar.dma_start(out=w2[:], in_=w_d[D:, :])
            xta = constp.tile([P, RPAD], bf16)
            for q in range(4):
                nc.scalar.dma_start(out=xta[:, q * 3200:(q + 1) * 3200],
                                    in_=xT_d[:, q * 3200:(q + 1) * 3200])

            arena = constp.tile([P, RPAD], bf16)
            # groups only cover [0, NG*128); memset the tail once
            nc.vector.memset(arena[:, NG * P:], 0.0)

            # pre-zero the gather buffers: trailing-negative pad slots are
            # skipped by descgen and would otherwise read uninitialized SBUF
            # (NaN * 0 = NaN in the matmul)
            for b in range(12):
                mv = mvp.tile([P, CALL // P, D], bf16, tag="mv")
                nc.vector.memset(mv[:], 0.0)

            ncalls = [(int(sk[k]) + CALL - 1) // CALL for k in range(NCHUNK)]
            callidx = [0, 0, 0, 0]
            mvtiles = [dict() for _ in range(NCHUNK)]
            smtiles = [None] * NCHUNK
            smlo = [0] * NCHUNK
            qrr = 0

            for g in range(NG):
                m = g // MBLK
                if g % MBLK == 0:
                    for k in range(NCHUNK):
                        lo, hi = mwin(m, k)
                        sm = smp.tile([P, mega_max], fp8, tag=f"sm{k}")
                        nc.sync.dma_start(
                            out=sm[:, :hi - lo],
                            in_=s_d[:, int(go[k]) + lo:int(go[k]) + hi])
                        smtiles[k], smlo[k] = sm, lo

                # issue gather calls covering this group's cells
                for k in range(NCHUNK):
                    need = int(cell_off[k, g] + caps[k, g])
                    while callidx[k] * CALL < need:
                        w = callidx[k]
                        nn = min(CALL, int(sk[k]) - w * CALL)
                        nn = ((nn + 127) // 128) * 128
                        mv = mvp.tile([P, CALL // P, D], bf16, tag="mv")
                        b0 = int(go[k]) + w * CALL
                        nc.gpsimd.dma_gather(
                            mv[:, :nn // P, :],
                            x_d[k * CHUNK:(k + 1) * CHUNK, :],
                            gi_t[:, b0 // 16:(b0 + nn) // 16], nn, nn, D,
                            queue_num=qrr % 4)
                        qrr += 1
                        nc.vector.tensor_tensor(
                            out=mv[:, :nn // P, :], in0=mv[:, :nn // P, :],
                            in1=val_t[:, b0 // P:(b0 + nn) // P, None
                                      ].to_broadcast([P, nn // P, D]),
                            op=mybir.AluOpType.mult)
                        mvtiles[k][w] = mv
                        if w >= 3:
                            mvtiles[k].pop(w - 3, None)
                        callidx[k] += 1

                psg = segp.tile([P, P], f32, tag="ps")
                nsub_tot = int(caps[:, g].sum()) // P
                sdone = 0
                for k in range(NCHUNK):
                    for s in range(int(caps[k, g]) // P):
                        q = int(cell_off[k, g]) + s * P
                        mv = mvtiles[k][q // CALL]
                        nc.tensor.matmul(
                            psg[:], mv[:, (q % CALL) // P, :],
                            smtiles[k][:, q - smlo[k]:q - smlo[k] + P],
                            start=(sdone == 0), stop=(sdone == nsub_tot - 1))
                        sdone += 1
                nc.scalar.activation(arena[:, g * P:(g + 1) * P], psg[:], copyf)

                if g % 4 == 3:
                    c0 = (g // 4) * 512
                    ph = headp.tile([P, 512], f32, tag="ph")
                    nc.tensor.matmul(ph[:], w1[:], arena[:, c0:c0 + 512],
                                     start=True, stop=False)
                    nc.tensor.matmul(ph[:], w2[:], xta[:, c0:c0 + 512],
                                     start=False, stop=True)
                    ob = outp.tile([P, 512], bf16, tag="ob")
                    nc.scalar.activation(ob[:], ph[:], relu)
                    nc.scalar.dma_start(out=outT_d[:, c0:c0 + 512], in_=ob[:])

            # final head batch: groups 96..97 plus zero tail (12288..12800)
            c0 = (NG // 4) * 512
            assert c0 == 12288
            ph = headp.tile([P, 512], f32, tag="ph")
            nc.tensor.matmul(ph[:], w1[:], arena[:, c0:c0 + 512],
                             start=True, stop=False)
            nc.tensor.matmul(ph[:], w2[:], xta[:, c0:c0 + 512],
                             start=False, stop=True)
            ob = outp.tile([P, 512], bf16, tag="ob")
            nc.scalar.activation(ob[:], ph[:], relu)
            nc.scalar.dma_start(out=outT_d[:, c0:c0 + 512], in_=ob[:])

    nc.compile()
    return nc


def _get_nc(key):
    nc = _compiled.get(key)
    if nc is None:
        nc = _build(key)
        _compiled[key] = nc
    return nc


def _make_in_maps(x, W, gi_w, s_w, v_w):
    x = np.asarray(x, np.float32)
    xb = x.astype(BF16)
    Wb = np.ascontiguousarray(np.asarray(W, np.float32).astype(BF16))
    in_maps = []
    for c in range(NCORES):
        xloc = np.zeros((D, RPAD), BF16)
        xloc[:, :RPC] = xb[c * RPC:(c + 1) * RPC].T
        in_maps.append({
            "x": xb,
            "xlocT": np.ascontiguousarray(xloc),
            "W": Wb,
            "gidx": gi_w[c],
            "smat": s_w[c],
            "vals": v_w[c],
        })
    return in_maps


def _install_trace_shims():
    """Make trace=True work in this container: provide antenv.axon_hooks
    (ctypes NTFF profiling via the axon PJRT .so) and stub the artifact
    upload (no bucket access here)."""
    import contextlib
    import ctypes
    import types

    try:
        import antenv.axon_hooks  # noqa: F401
        has_hooks = True
    except ImportError:
        has_hooks = False
    if not has_hooks:
        so_path = "/opt/axon/libaxon_pjrt.so"
        lib = ctypes.CDLL(so_path)
        if hasattr(lib, "axon_start_nrt_profile"):
            lib.axon_start_nrt_profile.argtypes = [
                ctypes.POINTER(ctypes.c_int64), ctypes.c_size_t]
            lib.axon_start_nrt_profile.restype = ctypes.c_int64
            lib.axon_stop_nrt_profile.argtypes = [ctypes.c_char_p]
            lib.axon_stop_nrt_profile.restype = ctypes.c_int64

            @contextlib.contextmanager
            def _hook(output_dir, device_ids):
                import jax
                jax.devices()
                if device_ids:
                    ids = (ctypes.c_int64 * len(device_ids))(*device_ids)
                    rc = lib.axon_start_nrt_profile(ids, len(device_ids))
                else:
                    rc = lib.axon_start_nrt_profile(None, 0)
                if rc != 0:
                    raise RuntimeError(f"axon_start_nrt_profile rc={rc}")
                try:
                    yield
                finally:
                    n = lib.axon_stop_nrt_profile(str(output_dir).encode())
                    if n <= 0:
                        print(f"ntff profile: rc={n} (no files?) at {output_dir}")

            mod = types.ModuleType("antenv.axon_hooks")
            mod.get_axon_ntff_profile_hook = lambda: _hook
            mod.set_axon_ntff_profile_hook = lambda h: None
            sys.modules["antenv.axon_hooks"] = mod

    import concourse.bass_utils as bu
    bu.upload_artifacts = lambda tmpdir: f"local:{tmpdir}"


def _run(x, adj_rows, adj_cols, adj_vals, W, trace=False):
    from concourse.bass_utils import run_bass_kernel_spmd
    if trace:
        try:
            _install_trace_shims()
        except Exception as e:  # tracing is best-effort
            print("trace shim install failed:", e)
    key, gi_w, s_w, v_w = _prep(adj_rows, adj_cols, adj_vals)
    nc = _get_nc(key)
    in_maps = _make_in_maps(x, W, gi_w, s_w, v_w)
    res = run_bass_kernel_spmd(nc, in_maps, list(range(NCORES)), trace=trace)
    out = np.concatenate(
        [np.asarray(res.results[c]["outT"])[:, :RPC].T.astype(np.float32)
         for c in range(NCORES)],
        axis=0)
    return np.ascontiguousarray(out, dtype=np.float32), res


def kernel(x, adj_rows, adj_cols, adj_vals, W):
    out, _ = _run(x, adj_rows, adj_cols, adj_vals, W, trace=False)
    return out


# revision 14
# speedup vs baseline: 1.4526x; 1.1893x over previous
"""Distributed Trainium2 kernel for GNN message passing (COO SpMM + dense head).

out = relu((A @ x) @ W[:128] + x @ W[128:])   with A given as COO (rows, cols, vals)

Strategy (8 NeuronCores, SPMD single graph):
  - Rows (destinations) sharded across cores: core c owns rows [c*12500, (c+1)*12500).
  - x replicated to every core's DRAM (bf16) via its input map; no collectives.
  - SpMM = hardware gather + SEGMENT-SUM VIA TENSOR-ENGINE MATMULS (no SWDGE
    scatter at all -- scatter-add descriptor generation was the baseline's
    dominant Pool-engine cost at ~6ns/descriptor):
      * edges sorted by (col-chunk k, row-group g = r//128, row); per (k,g)
        cell the edge count is padded to a shared 128-aligned capacity across
        cores (SPMD: one program). Pad slots hold gather idx -1 (skipped when
        trailing in a call) or 0 (mid-call; S column is zero either way).
      * gather calls are 1024-slot windows of each chunk's stream -- per-call
        fixed overhead on the Q7 descgen cores is ~600ns, so few big calls
        (98) beat per-cell calls (392).
      * per 128-edge subtile: one bf16 matmul  psum_g += msgs^T @ S_sub where
        S_sub[i, j] = val_i * onehot(r_i - 128g == j) is HOST-precomputed bf16
        (values folded in -> no vector work in the inner loop). psum_g
        accumulates the whole group's 4 chunk-cells (cells located inside the
        big gather windows by static offset arithmetic), then one scalar
        activation copies it (cast bf16) into an SBUF-resident hT arena
        [128 feat x 12800 rows]. h never touches DRAM.
  - Dense head overlapped with SpMM: every 4 groups, outT = relu(W1^T @ hT +
    W2^T @ xT) with N=512 matmuls (W stationary), relu on ScalarE, contiguous
    store of outT [128 x 12800]; host transposes at the end.
"""

import sys

if "/opt/trn_rl_repo" not in sys.path:
    sys.path.insert(0, "/opt/trn_rl_repo")

import numpy as np
import ml_dtypes

BF16 = ml_dtypes.bfloat16
FP8 = ml_dtypes.float8_e4m3

N_NODES = 100000
N_EDGES = 600000
D = 128
OUT = 128
P = 128
NCORES = 8
RPC = N_NODES // NCORES          # 12500 rows per core
NCHUNK = 4
CHUNK = N_NODES // NCHUNK        # 25000 (< 32768 so int16 gather idx works)
NG = (RPC + P - 1) // P          # 98 row-groups of 128 rows
RPAD = 12800                     # 25 head batches x 512 rows
CALL = 1024                      # gather slots per SWDGE call (ring limit)

_compiled = {}


def _prep(adj_rows, adj_cols, adj_vals):
    """Per-core uniform-shape gather idx + per-matmul segment-matrix streams.

    Edges of core c sorted by (chunk=col//25000, r). Cell (k,g) capacity =
    EXACT max edge count over cores (no 128 rounding) -> ~19% fewer gather
    slots/descriptors. Subtiles live on the fixed 128-slot grid of each
    stream; a subtile overlapping two cells feeds two matmuls. S is stored
    PER MATMUL j: S[i%128, j*128 + (r-128g)] = 1, zeros outside the cell's
    partition span (so lhsT is always the full 128-partition subtile).
    """
    rows = np.asarray(adj_rows).astype(np.int64)
    cols = np.asarray(adj_cols).astype(np.int64)
    vals = np.asarray(adj_vals).astype(np.float32)

    per_core = []
    counts = np.zeros((NCORES, NCHUNK * NG), np.int64)
    for c in range(NCORES):
        m = (rows >= c * RPC) & (rows < (c + 1) * RPC)
        r = rows[m] - c * RPC
        co = cols[m]
        v = vals[m]
        ch = co // CHUNK
        o = np.lexsort((r, ch))
        r, co, v, ch = r[o], co[o], v[o], ch[o]
        cell = ch * NG + (r >> 7)
        counts[c] = np.bincount(cell, minlength=NCHUNK * NG)
        per_core.append((r, co, v, cell))

    caps = counts.max(axis=0).reshape(NCHUNK, NG)       # [k, g] exact
    sk = caps.sum(axis=1)                               # stream k size
    go = np.concatenate([[0], np.cumsum(((sk + CALL - 1) // CALL) * CALL)])
    T = int(go[-1])
    cell_off = np.zeros((NCHUNK, NG), np.int64)
    cell_off[:, 1:] = np.cumsum(caps, axis=1)[:, :-1]
    gcell_off = (cell_off + go[:-1, None]).reshape(-1)

    # matmul schedule: per cell, #subtiles it intersects; j ids consecutive
    # per cell, cells ordered (k, g)
    s_lo = cell_off // 128
    s_hi = (cell_off + np.maximum(caps, 1) - 1) // 128
    mm_cnt = np.where(caps > 0, s_hi - s_lo + 1, 0)     # [k, g]
    mm_off = np.zeros(NCHUNK * NG + 1, np.int64)
    mm_off[1:] = np.cumsum(mm_cnt.reshape(-1))
    NMM = int(mm_off[-1])

    gi_w = np.zeros((NCORES, P, T // 16), np.int16)
    s_w = np.zeros((NCORES, P, NMM * P), FP8)
    v_w = np.zeros((NCORES, P, T // P), BF16)
    for c in range(NCORES):
        r, co, v, cell = per_core[c]
        n = len(r)
        starts = np.concatenate([[0], np.cumsum(counts[c])])
        pos = gcell_off[cell] + (np.arange(n) - starts[cell])
        q = pos - go[cell // NG]                        # stream-local slot
        gi = np.zeros(T, np.int16)
        for k in range(NCHUNK):                 # trailing pads of each stream
            gi[int(go[k]) + int(sk[k]):int(go[k + 1])] = -1
        gi[pos] = (co % CHUNK).astype(np.int16)
        gi_w[c] = np.tile(gi.reshape(-1, 16).T, (8, 1))
        j = mm_off[cell] + (q // P - s_lo.reshape(-1)[cell])
        S = np.zeros((P, NMM * P), np.float32)
        S[pos % P, j * P + (r - ((r >> 7) << 7))] = 1.0
        s_w[c] = S.astype(FP8)
        V = np.zeros((P, T // P), np.float32)
        V[pos % P, pos // P] = v
        v_w[c] = V.astype(BF16)

    key = tuple(int(x) for x in caps.reshape(-1))
    return key, gi_w, s_w, v_w


def _build(key):
    from concourse import bass, mybir, tile, bacc

    f32 = mybir.dt.float32
    bf16 = mybir.dt.bfloat16
    i16 = mybir.dt.int16
    fp8 = mybir.dt.float8e4
    relu = mybir.ActivationFunctionType.Relu
    copyf = mybir.ActivationFunctionType.Copy

    caps = np.asarray(key, np.int64).reshape(NCHUNK, NG)
    sk = caps.sum(axis=1)
    go = np.concatenate([[0], np.cumsum(((sk + CALL - 1) // CALL) * CALL)])
    T = int(go[-1])
    cell_off = np.zeros((NCHUNK, NG), np.int64)
    cell_off[:, 1:] = np.cumsum(caps, axis=1)[:, :-1]
    s_lo = cell_off // 128
    s_hi = (cell_off + np.maximum(caps, 1) - 1) // 128
    mm_cnt = np.where(caps > 0, s_hi - s_lo + 1, 0)
    mm_off = np.zeros(NCHUNK * NG + 1, np.int64)
    mm_off[1:] = np.cumsum(mm_cnt.reshape(-1))
    NMM = int(mm_off[-1])

    # S mega-tile windows: matmul-id ranges per (MBLK-group block m, stream k)
    MBLK = 4
    nmega = (NG + MBLK - 1) // MBLK
    def mwin(m, k):
        ge = min((m + 1) * MBLK, NG) - 1
        jlo = int(mm_off[k * NG + m * MBLK])
        jhi = int(mm_off[k * NG + ge] + mm_cnt[k, ge])
        return jlo, jhi
    mega_max = max((mwin(m, k)[1] - mwin(m, k)[0]) * P
                   for m in range(nmega) for k in range(NCHUNK))

    nc = bacc.Bacc("TRN2", target_bir_lowering=False, debug=False,
                   num_swdge_queues=4)

    x_d = nc.dram_tensor("x", [N_NODES, D], bf16, kind="ExternalInput")
    xT_d = nc.dram_tensor("xlocT", [D, RPAD], bf16, kind="ExternalInput")
    w_d = nc.dram_tensor("W", [2 * D, OUT], bf16, kind="ExternalInput")
    gi_d = nc.dram_tensor("gidx", [P, T // 16], i16, kind="ExternalInput")
    s_d = nc.dram_tensor("smat", [P, NMM * P], fp8, kind="ExternalInput")
    v_d = nc.dram_tensor("vals", [P, T // P], bf16, kind="ExternalInput")
    outT_d = nc.dram_tensor("outT", [OUT, RPAD], bf16, kind="ExternalOutput")

    with tile.TileContext(nc) as tc:
        with tc.tile_pool(name="const", bufs=1) as constp, \
             tc.tile_pool(name="smega", bufs=8) as smp, \
             tc.tile_pool(name="mess", bufs=12) as mvp, \
             tc.tile_pool(name="outb", bufs=2) as outp, \
             tc.tile_pool(name="psseg", bufs=3, space="PSUM") as segp, \
             tc.tile_pool(name="pshead", bufs=2, space="PSUM") as headp:

            # gather idx arena (whole stream resident)
            gi_t = constp.tile([P, T // 16], i16)
            qn = T // 16 // 4
            for q in range(4):
                q0 = q * qn
                q1 = (q + 1) * qn if q < 3 else T // 16
                nc.sync.dma_start(out=gi_t[:, q0:q1], in_=gi_d[:, q0:q1])

            val_t = constp.tile([P, T // P], bf16)
            nc.sync.dma_start(out=val_t[:], in_=v_d[:])
            w1 = constp.tile([D, OUT], bf16)
            nc.scalar.dma_start(out=w1[:], in_=w_d[:D, :])
            w2 = constp.tile([D, OUT], bf16)
            nc.scalar.dma_start(out=w2[:], in_=w_d[D:, :])
            xta = constp.tile([P, RPAD], bf16)
            for q in range(4):
                nc.scalar.dma_start(out=xta[:, q * 3200:(q + 1) * 3200],
                                    in_=xT_d[:, q * 3200:(q + 1) * 3200])

            arena = constp.tile([P, RPAD], bf16)
            # groups only cover [0, NG*128); memset the tail once
            nc.vector.memset(arena[:, NG * P:], 0.0)

            # pre-zero the gather buffers: trailing-negative pad slots are
            # skipped by descgen and would otherwise read uninitialized SBUF
            # (NaN * 0 = NaN in the matmul)
            for b in range(12):
                mv = mvp.tile([P, CALL // P, D], bf16, tag="mv")
                nc.vector.memset(mv[:], 0.0)

            ncalls = [(int(sk[k]) + CALL - 1) // CALL for k in range(NCHUNK)]
            callidx = [0, 0, 0, 0]
            mvtiles = [dict() for _ in range(NCHUNK)]
            smtiles = [None] * NCHUNK
            smlo = [0] * NCHUNK
            qrr = 0

            for g in range(NG):
                m = g // MBLK
                if g % MBLK == 0:
                    for k in range(NCHUNK):
                        jlo, jhi = mwin(m, k)
                        sm = smp.tile([P, mega_max], fp8, tag=f"sm{k}")
                        if jhi > jlo:
                            nc.sync.dma_start(
                                out=sm[:, :(jhi - jlo) * P],
                                in_=s_d[:, jlo * P:jhi * P])
                        smtiles[k], smlo[k] = sm, jlo

                # issue gather calls covering this group's cells
                for k in range(NCHUNK):
                    need = int(cell_off[k, g] + caps[k, g])
                    while callidx[k] * CALL < need:
                        w = callidx[k]
                        nn = min(CALL, int(sk[k]) - w * CALL)
                        nn = ((nn + 127) // 128) * 128
                        mv = mvp.tile([P, CALL // P, D], bf16, tag="mv")
                        b0 = int(go[k]) + w * CALL
                        nc.gpsimd.dma_gather(
                            mv[:, :nn // P, :],
                            x_d[k * CHUNK:(k + 1) * CHUNK, :],
                            gi_t[:, b0 // 16:(b0 + nn) // 16], nn, nn, D,
                            queue_num=qrr % 4)
                        qrr += 1
                        nc.vector.tensor_tensor(
                            out=mv[:, :nn // P, :], in0=mv[:, :nn // P, :],
                            in1=val_t[:, b0 // P:(b0 + nn) // P, None
                                      ].to_broadcast([P, nn // P, D]),
                            op=mybir.AluOpType.mult)
                        mvtiles[k][w] = mv
                        if w >= 3:
                            mvtiles[k].pop(w - 3, None)
                        callidx[k] += 1

                psg = segp.tile([P, P], f32, tag="ps")
                nsub_tot = int(mm_cnt[:, g].sum())
                sdone = 0
                for k in range(NCHUNK):
                    if caps[k, g] == 0:
                        continue
                    for i in range(int(mm_cnt[k, g])):
                        s = int(s_lo[k, g]) + i
                        j = int(mm_off[k * NG + g]) + i
                        so = (j - smlo[k]) * P
                        mv = mvtiles[k][s * P // CALL]
                        nc.tensor.matmul(
                            psg[:], mv[:, (s * P % CALL) // P, :],
                            smtiles[k][:, so:so + P],
                            start=(sdone == 0), stop=(sdone == nsub_tot - 1))
                        sdone += 1
                nc.scalar.activation(arena[:, g * P:(g + 1) * P], psg[:], copyf)

                if g % 4 == 3:
                    c0 = (g // 4) * 512
                    ph = headp.tile([P, 512], f32, tag="ph")
                    nc.tensor.matmul(ph[:], w1[:], arena[:, c0:c0 + 512],
                                     start=True, stop=False)
                    nc.tensor.matmul(ph[:], w2[:], xta[:, c0:c0 + 512],
                                     start=False, stop=True)
                    ob = outp.tile([P, 512], bf16, tag="ob")
                    nc.scalar.activation(ob[:], ph[:], relu)
                    nc.scalar.dma_start(out=outT_d[:, c0:c0 + 512], in_=ob[:])

            # final head batch: groups 96..97 plus zero tail (12288..12800)
            c0 = (NG // 4) * 512
            assert c0 == 12288
            ph = headp.tile([P, 512], f32, tag="ph")
            nc.tensor.matmul(ph[:], w1[:], arena[:, c0:c0 + 512],
                             start=True, stop=False)
            nc.tensor.matmul(ph[:], w2[:], xta[:, c0:c0 + 512],
                             start=False, stop=True)
            ob = outp.tile([P, 512], bf16, tag="ob")
            nc.scalar.activation(ob[:], ph[:], relu)
            nc.scalar.dma_start(out=outT_d[:, c0:c0 + 512], in_=ob[:])

    nc.compile()
    return nc


def _get_nc(key):
    nc = _compiled.get(key)
    if nc is None:
        nc = _build(key)
        _compiled[key] = nc
    return nc


def _make_in_maps(x, W, gi_w, s_w, v_w):
    x = np.asarray(x, np.float32)
    xb = x.astype(BF16)
    Wb = np.ascontiguousarray(np.asarray(W, np.float32).astype(BF16))
    in_maps = []
    for c in range(NCORES):
        xloc = np.zeros((D, RPAD), BF16)
        xloc[:, :RPC] = xb[c * RPC:(c + 1) * RPC].T
        in_maps.append({
            "x": xb,
            "xlocT": np.ascontiguousarray(xloc),
            "W": Wb,
            "gidx": gi_w[c],
            "smat": s_w[c],
            "vals": v_w[c],
        })
    return in_maps


def _install_trace_shims():
    """Make trace=True work in this container: provide antenv.axon_hooks
    (ctypes NTFF profiling via the axon PJRT .so) and stub the artifact
    upload (no bucket access here)."""
    import contextlib
    import ctypes
    import types

    try:
        import antenv.axon_hooks  # noqa: F401
        has_hooks = True
    except ImportError:
        has_hooks = False
    if not has_hooks:
        so_path = "/opt/axon/libaxon_pjrt.so"
        lib = ctypes.CDLL(so_path)
        if hasattr(lib, "axon_start_nrt_profile"):
            lib.axon_start_nrt_profile.argtypes = [
                ctypes.POINTER(ctypes.c_int64), ctypes.c_size_t]
            lib.axon_start_nrt_profile.restype = ctypes.c_int64
            lib.axon_stop_nrt_profile.argtypes = [ctypes.c_char_p]
            lib.axon_stop_nrt_profile.restype = ctypes.c_int64

            @contextlib.contextmanager
            def _hook(output_dir, device_ids):
                import jax
                jax.devices()
                if device_ids:
                    ids = (ctypes.c_int64 * len(device_ids))(*device_ids)
                    rc = lib.axon_start_nrt_profile(ids, len(device_ids))
                else:
                    rc = lib.axon_start_nrt_profile(None, 0)
                if rc != 0:
                    raise RuntimeError(f"axon_start_nrt_profile rc={rc}")
                try:
                    yield
                finally:
                    n = lib.axon_stop_nrt_profile(str(output_dir).encode())
                    if n <= 0:
                        print(f"ntff profile: rc={n} (no files?) at {output_dir}")

            mod = types.ModuleType("antenv.axon_hooks")
            mod.get_axon_ntff_profile_hook = lambda: _hook
            mod.set_axon_ntff_profile_hook = lambda h: None
            sys.modules["antenv.axon_hooks"] = mod

    import concourse.bass_utils as bu
    bu.upload_artifacts = lambda tmpdir: f"local:{tmpdir}"


def _run(x, adj_rows, adj_cols, adj_vals, W, trace=False):
    from concourse.bass_utils import run_bass_kernel_spmd
    if trace:
        try:
            _install_trace_shims()
        except Exception as e:  # tracing is best-effort
            print("trace shim install failed:", e)
    key, gi_w, s_w, v_w = _prep(adj_rows, adj_cols, adj_vals)
    nc = _get_nc(key)
    in_maps = _make_in_maps(x, W, gi_w, s_w, v_w)
    res = run_bass_kernel_spmd(nc, in_maps, list(range(NCORES)), trace=trace)
    out = np.concatenate(
        [np.asarray(res.results[c]["outT"])[:, :RPC].T.astype(np.float32)
         for c in range(NCORES)],
        axis=0)
    return np.ascontiguousarray(out, dtype=np.float32), res


def kernel(x, adj_rows, adj_cols, adj_vals, W):
    out, _ = _run(x, adj_rows, adj_cols, adj_vals, W, trace=False)
    return out
